# revision 16
# baseline (speedup 1.0000x reference)
"""ChromaSelfAttention TRN2 kernel: head-parallel across 8 NeuronCores.

Each core computes 3 of the 24 heads end-to-end (qkv projection, per-head
RMS norm, attention, softmax, out-projection partial) and returns a
[2048, 3072] bf16 partial of the final output; the host sums the 8
partials in fp32 and adds the output bias.

All per-core inputs ship in ONE packed [128, NCOLS] bf16 DRAM tensor
(one input + one output buffer per core minimizes per-exec dispatch cost).

Numerics are bf16 end-to-end (fp8 was tried and rejected: with zero-mean
V the attention output inherits the element-wise quantization error of
probs/V/scores un-averaged, ~4-8% >> the 2% budget).

Layouts (per core, all SBUF tiles partition-major [128, ...]):
  x*T  : x^T as 24 k-tiles [128, 2048]  (host pre-transposed, bf16)
  Q^T/K^T : [128d, 3h, 2048L]  (d on partitions -> scores contract over d)
  V    : [128L, 16jt, 384d]    (L on partitions -> PV contracts over j)
  S^T  : [128j, 1024i] psum tiles; softmax sums via ones-matmul over j
  O^T  : [128d, 2048i] -> out-proj contracts over d(=head dims)
"""

import numpy as np
import ml_dtypes

BF16 = ml_dtypes.bfloat16

H, DH, D, L = 24, 128, 3072, 2048
NC = 8
HPC = H // NC          # heads per core = 3
W = HPC * DH           # per-core projection width = 384
KT = D // 128          # contraction k-tiles = 24
JT = L // 128          # key tiles = 16
NB = 4                 # query i-blocks of 512
IB = L // NB           # 512
EPS = 1e-6

# packed input column offsets (all bf16)
COLS_X = KT * L        # 49152
COLS_W = KT * W        # 9216
OFF_XQ = 0
OFF_XK = OFF_XQ + COLS_X
OFF_XV = OFF_XK + COLS_X
OFF_WQ = OFF_XV + COLS_X
OFF_WK = OFF_WQ + COLS_W
OFF_WV = OFF_WK + COLS_W
OFF_WO = OFF_WV + COLS_W
OFF_IC2 = OFF_WO + HPC * D
NCOLS = OFF_IC2 + 1

_PROG = None           # cached compiled program


def _build_program(reps=1):
    import concourse.tile as tile
    import concourse.mybir as mybir
    from concourse import bacc

    bf16 = mybir.dt.bfloat16
    nc = bacc.Bacc("TRN2", target_bir_lowering=False, debug=False)

    xin_d = nc.dram_tensor("xin", [128, NCOLS], bf16, kind="ExternalInput")
    out_d = nc.dram_tensor("out", [L, D], bf16, kind="ExternalOutput")

    with tile.TileContext(nc) as tc:
        for _rep in range(reps):
            _emit_body(nc, tc, _rep, xin_d, out_d)

    nc.compile()
    return nc


def _emit_body(nc, tc, R, xin_d, out_d):
    import concourse.mybir as mybir
    from contextlib import ExitStack

    f32 = mybir.dt.float32
    bf16 = mybir.dt.bfloat16
    f32r = mybir.dt.float32r
    AF = mybir.ActivationFunctionType
    xin = xin_d.ap()

    with ExitStack() as ctx:
        consts = ctx.enter_context(tc.tile_pool(name=f"consts{R}", bufs=1))
        wop = ctx.enter_context(tc.tile_pool(name=f"wop{R}", bufs=1))
        otp = ctx.enter_context(tc.tile_pool(name=f"otp{R}", bufs=1))
        qkv = ctx.enter_context(tc.tile_pool(name=f"qkv{R}", bufs=1))

        ones_row = consts.tile([128, 128], bf16, tag="ones")
        nc.vector.memset(ones_row, 1.0)
        ones_sb = ones_row[:, 0:1]
        ic2_sb = consts.tile([128, 1], bf16, tag="ic2")
        nc.gpsimd.dma_start(out=ic2_sb, in_=xin[:, OFF_IC2:OFF_IC2 + 1])
        eps_sb = consts.tile([128, 1], f32, tag="eps")
        nc.vector.memset(eps_sb, EPS)
        ones32_f = consts.tile([128, 128], f32, tag="ones32")
        nc.vector.memset(ones32_f, 1.0)
        ones32 = ones32_f.bitcast(f32r)

        q_sb = qkv.tile([128, HPC, L], bf16, tag="qsb")
        k_sb = qkv.tile([128, HPC, L], bf16, tag="ksb")
        v_sb = qkv.tile([128, JT, W], bf16, tag="vsb")
        # O^T per head, kept until the out-projection
        ot_sb = [otp.tile([128, L], bf16, tag=f"otsb{h}", name=f"{R}_otsb{h}")
                 for h in range(HPC)]

        # ---------------- Phase A: projections ----------------
        with tc.tile_pool(name=f"wqkv{R}", bufs=1) as wp, \
             tc.tile_pool(name=f"xs{R}", bufs=3) as xs, \
             tc.tile_pool(name=f"psA{R}", bufs=8, space="PSUM") as psA:
            w_sbs = {}
            for name, off in (("wq", OFF_WQ), ("wk", OFF_WK), ("wv", OFF_WV)):
                t = wp.tile([128, KT, W], bf16, tag=name)
                for ch in range(4):
                    nc.scalar.dma_start(
                        out=t[:, ch * 6:(ch + 1) * 6, :],
                        in_=xin[:, off + ch * 6 * W: off + (ch + 1) * 6 * W].rearrange(
                            "p (kt w) -> p kt w", kt=6))
                w_sbs[name] = t
            wo_sb = wop.tile([128, HPC, D], bf16, tag="wo")
            nc.scalar.dma_start(
                out=wo_sb,
                in_=xin[:, OFF_WO: OFF_WO + HPC * D].rearrange("p (h c) -> p h c", h=HPC))

            # Q^T and K^T: [384, 2048] as 3 n-tiles; two column halves
            for wname, xoff, dst in (("wq", OFF_XQ, q_sb), ("wk", OFF_XK, k_sb)):
                wt = w_sbs[wname]
                for half in range(2):
                    pts = [[psA.tile([128, 512], f32, tag="pj",
                                     name=f"{R}_pj_{wname}_{half}_{n}_{m}")
                            for m in range(2)] for n in range(HPC)]
                    for kt in range(0, KT, 2):
                        xt = xs.tile([128, 2, 1024], bf16, tag="xqk")
                        nc.sync.dma_start(
                            out=xt,
                            in_=xin[:, xoff:xoff + COLS_X].rearrange(
                                "p (kt l) -> p kt l", kt=KT)[
                                :, kt:kt + 2, half * 1024:(half + 1) * 1024])
                        for kti in range(2):
                            for n in range(HPC):
                                for m in range(2):
                                    nc.tensor.matmul(
                                        pts[n][m],
                                        lhsT=wt[:, kt + kti, n * 128:(n + 1) * 128],
                                        rhs=xt[:, kti, m * 512:(m + 1) * 512],
                                        start=(kt + kti == 0),
                                        stop=(kt + kti == KT - 1))
                    for n in range(HPC):
                        for m in range(2):
                            dcol = half * 1024 + m * 512
                            dst_ap = dst[:, n, dcol:dcol + 512]
                            if (n + m) % 2 == 0:
                                nc.scalar.copy(out=dst_ap, in_=pts[n][m])
                            else:
                                nc.vector.tensor_copy(out=dst_ap, in_=pts[n][m])

            # V natural: 16 m-tiles [128, 384] in 3 groups
            wt = w_sbs["wv"]
            for ms in ((0, 1, 2, 3, 4, 5), (6, 7, 8, 9, 10, 11), (12, 13, 14, 15)):
                pvs = [psA.tile([128, W], f32, tag="pj", name=f"{R}_pjv_{ms[0]}_{i}")
                       for i in range(len(ms))]
                for kt in range(0, KT, 2):
                    xt = xs.tile([128, 2, len(ms) * 128], bf16, tag="xv")
                    nc.sync.dma_start(
                        out=xt,
                        in_=xin[:, OFF_XV:OFF_XV + COLS_X].rearrange(
                            "p (kt l) -> p kt l", kt=KT)[
                            :, kt:kt + 2, ms[0] * 128:(ms[-1] + 1) * 128])
                    for kti in range(2):
                        for i in range(len(ms)):
                            nc.tensor.matmul(
                                pvs[i],
                                lhsT=xt[:, kti, i * 128:(i + 1) * 128],
                                rhs=wt[:, kt + kti, :],
                                start=(kt + kti == 0), stop=(kt + kti == KT - 1))
                for i, m in enumerate(ms):
                    nc.vector.tensor_copy(out=v_sb[:, m, :], in_=pvs[i])

        # ---------------- Phases B+C: norms + attention ----------------
        with tc.tile_pool(name=f"sq{R}", bufs=2) as sq, \
             tc.tile_pool(name=f"rowv{R}", bufs=4) as rowv, \
             tc.tile_pool(name=f"bcast{R}", bufs=2) as bcp, \
             tc.tile_pool(name=f"qnp{R}", bufs=3) as qnp, \
             tc.tile_pool(name=f"expp{R}", bufs=34) as expp, \
             tc.tile_pool(name=f"psBC{R}", bufs=2, space="PSUM") as psBC:

            for h in range(HPC):
                # ---- head RMS norms ----
                q2 = sq.tile([128, L], bf16, tag="q2")
                nc.vector.tensor_mul(q2, q_sb[:, h, :], q_sb[:, h, :])
                k2 = sq.tile([128, L], bf16, tag="k2")
                nc.vector.tensor_mul(k2, k_sb[:, h, :], k_sb[:, h, :])

                qsum = psBC.tile([128, 512], f32, tag="sm", bufs=2, name=f"{R}_qsum{h}")
                for b in range(NB):
                    nc.tensor.matmul(qsum[32 * b:32 * b + 1, :], lhsT=ic2_sb,
                                     rhs=q2[:, b * IB:(b + 1) * IB],
                                     start=True, stop=True,
                                     tile_position=(0, 32 * b))
                ksum = psBC.tile([128, JT], f32, tag="sm", bufs=2, name=f"{R}_ksum{h}")
                for jt in range(JT):
                    nc.tensor.matmul(ksum[:, jt:jt + 1],
                                     lhsT=k2[:, jt * 128:(jt + 1) * 128],
                                     rhs=ones_sb, start=True, stop=True)

                rk = rowv.tile([128, JT], f32, tag="rk", bufs=3)
                nc.scalar.activation(rk, ksum, AF.Sqrt, bias=eps_sb, scale=1.0 / DH)
                nc.vector.reciprocal(rk, rk)

                rqrow = rowv.tile([128, 512], f32r, tag="rqrow")
                with nc.allow_low_precision(reason="f32r rows feed broadcast matmul; fp32 bits"):
                    for b in range(NB):
                        nc.scalar.activation(rqrow[32 * b:32 * b + 1, :],
                                             qsum[32 * b:32 * b + 1, :],
                                             AF.Sqrt, bias=eps_sb[32 * b:32 * b + 1, :],
                                             scale=1.0 / DH)
                        nc.vector.reciprocal(rqrow[32 * b:32 * b + 1, :],
                                             rqrow[32 * b:32 * b + 1, :])
                qn = qnp.tile([128, L], bf16, tag="qn")
                for b in range(NB):
                    rq_bc = psBC.tile([128, 512], f32, tag="sm", bufs=2,
                                      name=f"{R}_rqbc_{h}_{b}")
                    nc.tensor.matmul(rq_bc, lhsT=ones32[32 * b:32 * b + 1, :],
                                     rhs=rqrow[32 * b:32 * b + 1, :],
                                     start=True, stop=True,
                                     tile_position=(32 * b, 0))
                    nc.vector.tensor_mul(qn[:, b * IB:(b + 1) * IB],
                                         q_sb[:, h, b * IB:(b + 1) * IB], rq_bc)

                # ---- attention: two 1024-wide query blocks ----
                sums = psBC.tile([128, 512], f32, tag="sm", bufs=2, name=f"{R}_sums{h}")
                den_row = rowv.tile([128, 512], f32r, tag="den", name=f"{R}_den{h}")
                for bb in range(2):
                    ets = []
                    otps = psBC.tile([128, 1024], f32, tag="ot", bufs=1,
                                     name=f"{R}_otps_{h}_{bb}")
                    for jt in range(JT):
                        scp = psBC.tile([128, 1024], f32, tag="sc", bufs=2,
                                        name=f"{R}_scp_{h}_{bb}_{jt}")
                        for hf in range(2):
                            nc.tensor.matmul(
                                scp[:, hf * 512:(hf + 1) * 512],
                                lhsT=k_sb[:, h, jt * 128:(jt + 1) * 128],
                                rhs=qn[:, bb * 1024 + hf * 512: bb * 1024 + (hf + 1) * 512],
                                start=True, stop=True)
                        et = expp.tile([128, 1024], bf16, tag="exp",
                                       name=f"{R}_exp_{h}_{bb}_{jt}")
                        nc.scalar.activation(et, scp, AF.Exp, scale=rk[:, jt:jt + 1])
                        ets.append(et)
                    for jt in range(JT):
                        for hf in range(2):
                            nc.tensor.matmul(
                                otps[:, hf * 512:(hf + 1) * 512],
                                lhsT=v_sb[:, jt, h * 128:(h + 1) * 128],
                                rhs=ets[jt][:, hf * 512:(hf + 1) * 512],
                                start=(jt == 0), stop=(jt == JT - 1))
                    for jt in range(JT):
                        for hf in range(2):
                            b = bb * 2 + hf
                            nc.tensor.matmul(
                                sums[32 * b:32 * b + 1, :],
                                lhsT=ones_sb, rhs=ets[jt][:, hf * 512:(hf + 1) * 512],
                                start=(jt == 0), stop=(jt == JT - 1),
                                tile_position=(0, 32 * b))
                    for hf in range(2):
                        b = bb * 2 + hf
                        with nc.allow_low_precision(reason="f32r row feeds broadcast matmul; fp32 bits"):
                            nc.vector.reciprocal(den_row[32 * b:32 * b + 1, :],
                                                 sums[32 * b:32 * b + 1, :])
                        den_bc = psBC.tile([128, 512], f32, tag="sm", bufs=2,
                                           name=f"{R}_denbc_{h}_{b}")
                        nc.tensor.matmul(den_bc, lhsT=ones32[32 * b:32 * b + 1, :],
                                         rhs=den_row[32 * b:32 * b + 1, :],
                                         start=True, stop=True,
                                         tile_position=(32 * b, 0))
                        otraw = bcp.tile([128, 512], bf16, tag="otraw", bufs=2,
                                         name=f"{R}_otraw_{h}_{b}")
                        nc.vector.tensor_copy(out=otraw, in_=otps[:, hf * 512:(hf + 1) * 512])
                        nc.vector.tensor_mul(ot_sb[h][:, b * IB:(b + 1) * IB],
                                             otraw, den_bc)

            # ---- out-projection: shares the "sc" psum slots so it
            # overlaps the tail of the attention phase ----
            for it in range(JT):
                osb = bcp.tile([128, D], bf16, tag="osb", bufs=3,
                               name=f"{R}_osb{it}")
                for c in range(6):
                    od = psBC.tile([128, 512], f32, tag="sc", bufs=2,
                                   name=f"{R}_od_{it}_{c}")
                    for h in range(HPC):
                        nc.tensor.matmul(od,
                                         lhsT=ot_sb[h][:, it * 128:(it + 1) * 128],
                                         rhs=wo_sb[:, h, c * 512:(c + 1) * 512],
                                         start=(h == 0), stop=(h == HPC - 1))
                    if c % 2 == 0:
                        nc.scalar.copy(out=osb[:, c * 512:(c + 1) * 512], in_=od)
                    else:
                        nc.vector.tensor_copy(out=osb[:, c * 512:(c + 1) * 512], in_=od)
                nc.sync.dma_start(
                    out=out_d.ap()[it * 128:(it + 1) * 128, :], in_=osb)


def _pm(a2d, kt):
    """[kt*128, F] -> partition-major [128, kt*F], C-contiguous bf16."""
    f = a2d.shape[1]
    return np.ascontiguousarray(
        a2d.reshape(kt, 128, f).transpose(1, 0, 2).reshape(128, kt * f).astype(BF16))


def prepare_inputs(inputs):
    xq = np.asarray(inputs["x_q"], np.float32).reshape(L, D)
    xk = np.asarray(inputs["x_k"], np.float32).reshape(L, D)
    xv = np.asarray(inputs["x_v"], np.float32).reshape(L, D)
    Wq = np.asarray(inputs["Wq"], np.float32)
    Wk = np.asarray(inputs["Wk"], np.float32)
    Wv = np.asarray(inputs["Wv"], np.float32)
    Wo = np.asarray(inputs["Wo"], np.float32)
    nqw = np.asarray(inputs["norm_q_w"], np.float32)
    nkw = np.asarray(inputs["norm_k_w"], np.float32)

    c = nqw * nkw * (DH ** -0.5)                       # [128] per-head-dim scale
    assert np.all(c != 0.0)
    ic2 = (1.0 / (c * c)).astype(BF16).reshape(128, 1)
    cfull = np.tile(c, HPC)                            # [384]

    xqT = _pm(xq.T.astype(np.float32), KT)
    xkT = _pm(xk.T.astype(np.float32), KT)
    xvT = _pm(xv.T.astype(np.float32), KT)

    in_maps = []
    for i in range(NC):
        cols = slice(i * W, (i + 1) * W)
        xin = np.empty((128, NCOLS), BF16)
        xin[:, OFF_XQ:OFF_XQ + COLS_X] = xqT
        xin[:, OFF_XK:OFF_XK + COLS_X] = xkT
        xin[:, OFF_XV:OFF_XV + COLS_X] = xvT
        xin[:, OFF_WQ:OFF_WQ + COLS_W] = _pm(Wq[:, cols] * cfull[None, :], KT)
        xin[:, OFF_WK:OFF_WK + COLS_W] = _pm(Wk[:, cols], KT)
        xin[:, OFF_WV:OFF_WV + COLS_W] = _pm(Wv[:, cols], KT)
        xin[:, OFF_WO:OFF_WO + HPC * D] = _pm(Wo[cols, :], HPC)
        xin[:, OFF_IC2:OFF_IC2 + 1] = ic2
        in_maps.append({"xin": xin})
    return in_maps


def _bias_fallback(inputs):
    """Exact numpy path if projection biases are ever nonzero (they are all
    zero in this problem's setup_inputs, so this never runs)."""
    xq = np.asarray(inputs["x_q"], np.float32).reshape(L, D)
    xk = np.asarray(inputs["x_k"], np.float32).reshape(L, D)
    xv = np.asarray(inputs["x_v"], np.float32).reshape(L, D)
    q = xq @ np.asarray(inputs["Wq"]) + np.asarray(inputs["bq"])
    k = xk @ np.asarray(inputs["Wk"]) + np.asarray(inputs["bk"])
    v = xv @ np.asarray(inputs["Wv"]) + np.asarray(inputs["bv"])
    nqw = np.asarray(inputs["norm_q_w"], np.float32)
    nkw = np.asarray(inputs["norm_k_w"], np.float32)
    out = np.zeros((L, D), np.float32)
    for hh in range(H):
        qs, ks, vs = (t[:, hh * DH:(hh + 1) * DH] for t in (q, k, v))
        qn = qs / np.sqrt((qs ** 2).mean(-1, keepdims=True) + EPS) * nqw
        kn = ks / np.sqrt((ks ** 2).mean(-1, keepdims=True) + EPS) * nkw
        s = qn @ kn.T * DH ** -0.5
        p = np.exp(s - s.max(-1, keepdims=True))
        p /= p.sum(-1, keepdims=True)
        out += (p @ vs) @ np.asarray(inputs["Wo"])[hh * DH:(hh + 1) * DH, :]
    out = out + np.asarray(inputs["bo"], np.float32)[None, :]
    return out.reshape(1, L, D).astype(np.float32)


def kernel(**inputs):
    global _PROG
    bq = np.asarray(inputs["bq"], np.float32)
    bk = np.asarray(inputs["bk"], np.float32)
    bv = np.asarray(inputs["bv"], np.float32)
    if bq.any() or bk.any() or bv.any():
        return _bias_fallback(inputs)
    bo = np.asarray(inputs["bo"], np.float32)

    from concourse.bass_utils import run_bass_kernel_spmd

    if _PROG is None:
        _PROG = _build_program()
    in_maps = prepare_inputs(inputs)
    res = run_bass_kernel_spmd(_PROG, in_maps, core_ids=list(range(NC)))
    acc = np.zeros((L, D), np.float32)
    for r in res.results:
        acc += r["out"].astype(np.float32)
    out = (acc + bo[None, :]).astype(np.float32)
    return out.reshape(1, L, D)
